# revision 3
# baseline (speedup 1.0000x reference)
"""GAT layer (single head) on Trainium2, 8 NeuronCores — v2.

exp(leaky_relu(t)) = max(exp(t), exp(0.2 t)); each side separates into
per-src x per-dst factors. Host splits edges by sign(t) using phase-A
a-values, so each edge weight is (table-premultiplied per-src factor) x
(per-dst factor applied after aggregation).

Phase A: h = x@W; emits bf16 tables rowP=[exp(a_src)*h|exp(a_src)],
         rowM=[exp(.2 a_src)*h|exp(.2 a_src)] and per-node a/q values.
Host:    edges -> (dst-tile, bank, sign) sections, 128-edge blocks,
         shared max-over-cores schedule; int16 idx + bf16 dst-local
         streams (pad slots: dstl=-1 kills their contribution).
Phase B: dma_gather rows; per block one DVE is_equal one-hot + one PE
         matmul accumulating [dst,49] in PSUM (col 48 = denominator);
         per-tile q-scaling + affine self-loop term; batched ELU,
         linear, log_softmax.
"""
import numpy as np
import ml_dtypes

N_NODES = 100_000
N_EDGES = 1_600_000
IN_CH = 128
HIDDEN = 48
OUT_CH = 16
NEG_SLOPE = 0.2

P = 128
CORES = 8
NA = 12500                    # phase-A nodes per core
NT_A = 98                     # phase-A tiles per core
GT = 782                      # global dst tiles (781*128 + 32)
NT = 98                       # phase-B tile slots per core
ROWE = 128                    # table row elems (bf16) -> 256B
RU = 49                       # used row elems: 48 ch + denom
BANK_BASE = (32768, 98304)
KT = 8                        # tiles per gather batch
NQ = 4

_f32 = np.float32
_bf16 = ml_dtypes.bfloat16


# ---------------------------------------------------------------- phase A
def _build_phase_a():
    import concourse.bacc as bacc
    import concourse.mybir as mybir
    import concourse.tile as tile
    from concourse.masks import make_identity

    AL = mybir.AluOpType
    AF = mybir.ActivationFunctionType

    nc = bacc.Bacc("TRN2", target_bir_lowering=False, debug=False,
                   num_devices=CORES)
    xT = nc.dram_tensor("xT", [P, NT_A * P], mybir.dt.float32,
                        kind="ExternalInput")
    W = nc.dram_tensor("W", [IN_CH, HIDDEN], mybir.dt.float32,
                       kind="ExternalInput")
    att = nc.dram_tensor("att", [HIDDEN, 2], mybir.dt.float32,
                         kind="ExternalInput")
    rowP = nc.dram_tensor("rowP", [P, NT_A, RU], mybir.dt.bfloat16,
                          kind="ExternalOutput")
    rowM = nc.dram_tensor("rowM", [P, NT_A, RU], mybir.dt.bfloat16,
                          kind="ExternalOutput")
    avals = nc.dram_tensor("avals", [P, NT_A, 2], mybir.dt.float32,
                           kind="ExternalOutput")
    qvals = nc.dram_tensor("qvals", [P, NT_A, 2], mybir.dt.float32,
                           kind="ExternalOutput")

    NCHUNK = 7
    CH = NT_A // NCHUNK

    with tile.TileContext(nc) as tc:
        with (
            tc.tile_pool(name="const", bufs=1) as cp,
            tc.tile_pool(name="xp", bufs=2) as xp,
            tc.tile_pool(name="ps", bufs=2, space="PSUM") as ps,
            tc.tile_pool(name="ps2", bufs=2, space="PSUM") as ps2,
        ):
            ident = cp.tile([P, P], mybir.dt.float32)
            make_identity(nc, ident[:])
            w_sb = cp.tile([IN_CH, HIDDEN], mybir.dt.float32)
            nc.sync.dma_start(out=w_sb[:], in_=W[:, :])
            att_sb = cp.tile([HIDDEN, 2], mybir.dt.float32)
            nc.sync.dma_start(out=att_sb[:], in_=att[:, :])

            wT_ps = ps.tile([HIDDEN, IN_CH], mybir.dt.float32, space="PSUM")
            nc.tensor.transpose(out=wT_ps[:], in_=w_sb[:], identity=ident[:])
            wT_sb = cp.tile([HIDDEN, IN_CH], mybir.dt.float32)
            nc.vector.tensor_copy(out=wT_sb[:], in_=wT_ps[:])
            wa_ps = ps2.tile([P, 2], mybir.dt.float32, space="PSUM")
            nc.tensor.matmul(out=wa_ps[:], lhsT=wT_sb[:], rhs=att_sb[:],
                             start=True, stop=True)
            rhs50 = cp.tile([IN_CH, HIDDEN + 2], mybir.dt.float32)
            nc.vector.tensor_copy(out=rhs50[:, 0:HIDDEN], in_=w_sb[:])
            nc.vector.tensor_copy(out=rhs50[:, HIDDEN:HIDDEN + 2],
                                  in_=wa_ps[:])

            hstage = cp.tile([P, NT_A, HIDDEN + 2], mybir.dt.float32)
            asrc_t = hstage[:, :, HIDDEN]
            adst_t = hstage[:, :, HIDDEN + 1]

            Pt = cp.tile([P, NT_A], mybir.dt.float32)
            P2t = cp.tile([P, NT_A], mybir.dt.float32)
            qt = cp.tile([P, NT_A], mybir.dt.float32)
            q2t = cp.tile([P, NT_A], mybir.dt.float32)
            tmp = cp.tile([P, NT_A], mybir.dt.float32)
            rP = cp.tile([P, NT_A, RU], mybir.dt.bfloat16)
            rM = cp.tile([P, NT_A, RU], mybir.dt.bfloat16)

            GRP = 7   # tiles per PSUM tile: 7*50=350 <= 512 f32 bank
            for ck in range(NCHUNK):
                xt = xp.tile([P, CH * P], mybir.dt.float32, tag="xt")
                nc.sync.dma_start(out=xt[:],
                                  in_=xT[:, ck * CH * P:(ck + 1) * CH * P])
                for g0 in range(0, CH, GRP):
                    gn = min(GRP, CH - g0)
                    h_ps = ps.tile([P, GRP, HIDDEN + 2], mybir.dt.float32,
                                   space="PSUM", tag="hps")
                    for j in range(gn):
                        nc.tensor.matmul(out=h_ps[:, j, :],
                                         lhsT=xt[:, (g0 + j) * P:(g0 + j + 1) * P],
                                         rhs=rhs50[:], start=True, stop=True)
                    t = ck * CH + g0
                    nc.vector.tensor_copy(out=hstage[:, t:t + gn, :],
                                          in_=h_ps[:, 0:gn, :])
                # per-chunk exp factors + premultiplied rows
                ta, tb = ck * CH, (ck + 1) * CH
                nc.scalar.activation(out=Pt[:, ta:tb],
                                     in_=hstage[:, ta:tb, HIDDEN], func=AF.Exp)
                nc.vector.tensor_scalar(out=tmp[:, ta:tb],
                                        in0=hstage[:, ta:tb, HIDDEN],
                                        scalar1=NEG_SLOPE, scalar2=None,
                                        op0=AL.mult)
                nc.scalar.activation(out=P2t[:, ta:tb], in_=tmp[:, ta:tb],
                                     func=AF.Exp)
                nc.scalar.activation(out=qt[:, ta:tb],
                                     in_=hstage[:, ta:tb, HIDDEN + 1],
                                     func=AF.Exp)
                nc.vector.tensor_scalar(out=tmp[:, ta:tb],
                                        in0=hstage[:, ta:tb, HIDDEN + 1],
                                        scalar1=NEG_SLOPE, scalar2=None,
                                        op0=AL.mult)
                nc.scalar.activation(out=q2t[:, ta:tb], in_=tmp[:, ta:tb],
                                     func=AF.Exp)
                nc.vector.tensor_tensor(
                    out=rP[:, ta:tb, 0:HIDDEN], in0=hstage[:, ta:tb, 0:HIDDEN],
                    in1=Pt[:, ta:tb, None].broadcast_to([P, CH, HIDDEN]),
                    op=AL.mult)
                nc.vector.tensor_copy(out=rP[:, ta:tb, HIDDEN],
                                      in_=Pt[:, ta:tb])
                nc.vector.tensor_tensor(
                    out=rM[:, ta:tb, 0:HIDDEN], in0=hstage[:, ta:tb, 0:HIDDEN],
                    in1=P2t[:, ta:tb, None].broadcast_to([P, CH, HIDDEN]),
                    op=AL.mult)
                nc.vector.tensor_copy(out=rM[:, ta:tb, HIDDEN],
                                      in_=P2t[:, ta:tb])

            nc.sync.dma_start(out=rowP[:, :, :], in_=rP[:])
            nc.sync.dma_start(out=rowM[:, :, :], in_=rM[:])

            av = cp.tile([P, NT_A, 2], mybir.dt.float32)
            nc.vector.tensor_copy(out=av[:, :, 0], in_=asrc_t)
            nc.vector.tensor_copy(out=av[:, :, 1], in_=adst_t)
            nc.sync.dma_start(out=avals[:, :, :], in_=av[:])
            qv = cp.tile([P, NT_A, 2], mybir.dt.float32)
            nc.vector.tensor_copy(out=qv[:, :, 0], in_=qt[:])
            nc.vector.tensor_copy(out=qv[:, :, 1], in_=q2t[:])
            nc.sync.dma_start(out=qvals[:, :, :], in_=qv[:])

    nc.finalize()
    return nc


# ---------------------------------------------------------------- layout
def _layout2(src, dst, sign):
    """Shared-schedule edge layout. Returns schedule + per-core streams."""
    tg = dst >> 7                                     # global dst tile
    pl = (dst & 127).astype(np.int64)
    bank = (src >= 65536).astype(np.int64)
    sec = bank * 2 + (1 - sign.astype(np.int64))      # 0:b0+,1:b0-,2:b1+,3:b1-

    # per-(global tile, sec) counts -> blocks
    keyts = tg * 4 + sec
    cnt_t = np.bincount(keyts, minlength=GT * 4).reshape(GT, 4)
    nblk_t = (cnt_t + 127) >> 7                       # [GT, 4]

    # assign tiles to (core, slot): sort by block profile, deal rows of 8
    prof = nblk_t[:, 0] * 1000000 + nblk_t[:, 1] * 10000 \
        + nblk_t[:, 2] * 100 + nblk_t[:, 3]
    order = np.argsort(-prof, kind="stable")          # [GT]
    slot_of_tile = np.empty(GT, np.int64)
    core_of_tile = np.empty(GT, np.int64)
    for s in range((GT + CORES - 1) // CORES):
        grp = order[s * CORES:(s + 1) * CORES]
        slot_of_tile[grp] = s
        core_of_tile[grp] = np.arange(len(grp))
    NSLOT = (GT + CORES - 1) // CORES                 # 98
    assert NSLOT == NT

    # shared schedule: max blocks across the <=8 tiles of each slot
    nblk_sh = np.zeros((NT, 4), np.int64)
    for t in range(GT):
        s = slot_of_tile[t]
        nblk_sh[s] = np.maximum(nblk_sh[s], nblk_t[t])

    col_base = np.zeros((NT, 4), np.int64)
    calls = []                                        # dicts: sec, col0, cols
    col = 0
    NBATCH = (NT + KT - 1) // KT
    for bt in range(NBATCH):
        t0, t1 = bt * KT, min((bt + 1) * KT, NT)
        for s4 in range(4):
            ch = 0
            for t in range(t0, t1):
                col_base[t, s4] = col + ch
                ch += int(nblk_sh[t, s4])
            if ch:
                calls.append(dict(sec=s4, col0=col, cols=ch))
            col += ch
    total_cols = col

    # edge slot positions (within its (tile, sec) section, shared geometry)
    core = core_of_tile[tg]
    slot = slot_of_tile[tg]
    key = ((core * NT + slot) * 4 + sec)
    E = src.shape[0]
    order_e = np.lexsort((src, key))
    ks = key[order_e]
    change = np.r_[True, ks[1:] != ks[:-1]]
    gstart = np.where(change, np.arange(E), 0)
    gstart = np.maximum.accumulate(gstart)
    within = np.empty(E, np.int64)
    within[order_e] = np.arange(E) - gstart

    ecol = col_base[slot, sec] + (within >> 7)
    epos = ecol * P + (within & 127)
    biased = np.where(bank == 0, src - BANK_BASE[0], src - BANK_BASE[1])

    idx_streams, dstl_streams = [], []
    call_bounds = [(c["col0"] * P, (c["col0"] + c["cols"]) * P)
                   for c in calls]
    for c in range(CORES):
        idx = np.zeros(total_cols * P, np.int16)
        dstl = np.full((P, total_cols), -1.0, _bf16)
        m = core == c
        idx[epos[m]] = biased[m].astype(np.int16)
        dstl[(epos[m] & 127), (epos[m] >> 7)] = pl[m].astype(_bf16)
        blocks = [idx[a:b].reshape(-1, 16).T for a, b in call_bounds]
        w16 = np.concatenate(blocks, axis=1)
        idx_streams.append(np.tile(w16, (8, 1)).astype(np.int16))
        dstl_streams.append(dstl)

    node_of = np.full((CORES, NT, P), -1, np.int64)
    for t in range(GT):
        n0, n1 = t * P, min(t * P + P, N_NODES)
        node_of[core_of_tile[t], slot_of_tile[t], :n1 - n0] = \
            np.arange(n0, n1)

    return dict(calls=calls, col_base=col_base, nblk_sh=nblk_sh,
                total_cols=total_cols, idx=idx_streams, dstl=dstl_streams,
                node_of=node_of)


# ---------------------------------------------------------------- phase B
def _build_phase_b(calls, col_base, nblk_sh, total_cols):
    import concourse.bacc as bacc
    import concourse.mybir as mybir
    import concourse.tile as tile
    from concourse.masks import make_identity

    AL = mybir.AluOpType
    AF = mybir.ActivationFunctionType
    total16 = total_cols * P // 16

    nc = bacc.Bacc("TRN2", target_bir_lowering=False, debug=False,
                   num_devices=CORES, num_swdge_queues=NQ)
    tblP = nc.dram_tensor("tblP", [N_NODES, ROWE], mybir.dt.bfloat16,
                          kind="ExternalInput")
    tblM = nc.dram_tensor("tblM", [N_NODES, ROWE], mybir.dt.bfloat16,
                          kind="ExternalInput")
    idxs = nc.dram_tensor("idxs", [P, total16], mybir.dt.int16,
                          kind="ExternalInput")
    dstlt = nc.dram_tensor("dstl", [P, total_cols], mybir.dt.bfloat16,
                           kind="ExternalInput")
    qst = nc.dram_tensor("qst", [P, NT, 2], mybir.dt.float32,
                         kind="ExternalInput")
    selfR = nc.dram_tensor("selfR", [P, NT, RU], mybir.dt.bfloat16,
                           kind="ExternalInput")
    qefft = nc.dram_tensor("qeffh", [P, NT], mybir.dt.float32,
                           kind="ExternalInput")
    biasr = nc.dram_tensor("biasr", [P, HIDDEN], mybir.dt.float32,
                           kind="ExternalInput")
    linWt = nc.dram_tensor("linW", [HIDDEN, OUT_CH], mybir.dt.float32,
                           kind="ExternalInput")
    linbr = nc.dram_tensor("linbr", [P, OUT_CH], mybir.dt.float32,
                           kind="ExternalInput")
    outz = nc.dram_tensor("outz", [P, NT, OUT_CH], mybir.dt.float32,
                          kind="ExternalOutput")

    def win(s4):
        tbl = tblP if s4 in (0, 2) else tblM
        base = BANK_BASE[0] if s4 < 2 else BANK_BASE[1]
        return tbl[base:N_NODES, :]

    with tile.TileContext(nc) as tc:
        with (
            tc.tile_pool(name="const", bufs=1) as cp,
            tc.tile_pool(name="ix", bufs=2) as ixp,
            tc.tile_pool(name="g0", bufs=2) as gp0,
            tc.tile_pool(name="g1", bufs=2) as gp1,
            tc.tile_pool(name="g2", bufs=2) as gp2,
            tc.tile_pool(name="g3", bufs=2) as gp3,
            tc.tile_pool(name="m0a", bufs=2) as mp0,
            tc.tile_pool(name="m0b", bufs=2) as mp1,
            tc.tile_pool(name="m0c", bufs=2) as mp2,
            tc.tile_pool(name="m0d", bufs=2) as mp3,
            tc.tile_pool(name="sc", bufs=4) as sp,
            tc.tile_pool(name="big", bufs=1) as bigp,
            tc.tile_pool(name="pp", bufs=2, space="PSUM") as ppp,
            tc.tile_pool(name="pn", bufs=2, space="PSUM") as ppn,
            tc.tile_pool(name="py", bufs=2, space="PSUM") as pyp,
            tc.tile_pool(name="pz", bufs=2, space="PSUM") as pzp,
        ):
            ident = cp.tile([P, P], mybir.dt.float32)
            make_identity(nc, ident[:])
            ioi = sp.tile([P, P], mybir.dt.int32, tag="ioi")
            nc.gpsimd.iota(ioi[:], pattern=[[1, P]], base=0,
                           channel_multiplier=0)
            iota = cp.tile([P, P], mybir.dt.bfloat16)
            nc.vector.tensor_copy(out=iota[:], in_=ioi[:])

            dstl_sb = cp.tile([P, total_cols], mybir.dt.bfloat16)
            nc.sync.dma_start(out=dstl_sb[:], in_=dstlt[:, :])
            q_sb = cp.tile([P, NT, 2], mybir.dt.float32)
            nc.sync.dma_start(out=q_sb[:], in_=qst[:, :, :])
            bias_sb = cp.tile([P, HIDDEN], mybir.dt.float32)
            nc.sync.dma_start(out=bias_sb[:], in_=biasr[:, :])
            linW_sb = cp.tile([HIDDEN, OUT_CH], mybir.dt.float32)
            nc.sync.dma_start(out=linW_sb[:], in_=linWt[:, :])
            linb_sb = cp.tile([P, OUT_CH], mybir.dt.float32)
            nc.sync.dma_start(out=linb_sb[:], in_=linbr[:, :])

            rEff = cp.tile([P, NT, RU], mybir.dt.bfloat16)
            nc.sync.dma_start(out=rEff[:], in_=selfR[:, :, :])
            qeff = cp.tile([P, NT], mybir.dt.float32)
            nc.sync.dma_start(out=qeff[:], in_=qefft[:, :])

            aggbig = cp.tile([P, NT, RU], mybir.dt.float32)
            zst = cp.tile([P, NT, OUT_CH], mybir.dt.float32)

            gpools = {0: gp0, 1: gp1, 2: gp2, 3: gp3}
            mpools = {0: mp0, 1: mp1, 2: mp2, 3: mp3}

            def emit_sign(t, g, sgn, ntot, nbl, acc, tmp2, first):
                pool = ppp if sgn == 0 else ppn
                pst = pool.tile([P, RU], mybir.dt.float32, space="PSUM",
                                tag="pos" if sgn == 0 else "neg")
                done = 0
                for s4 in (sgn, sgn + 2):
                    for b in range(nbl[s4]):
                        gt, c0, chunks = g[s4]
                        rc = col_base[t, s4] + b - c0
                        m0b, mrow = chunks[rc]
                        nc.tensor.matmul(
                            out=pst[:], lhsT=m0b[:, mrow, :],
                            rhs=gt[:, rc, 0:RU],
                            start=(done == 0), stop=(done == ntot - 1))
                        done += 1
                qcol = 0 if sgn == 0 else 1
                if first:
                    nc.vector.tensor_scalar_mul(
                        out=acc[:], in0=pst[:],
                        scalar1=q_sb[:, t, qcol:qcol + 1])
                else:
                    nc.vector.tensor_scalar_mul(
                        out=tmp2[:], in0=pst[:],
                        scalar1=q_sb[:, t, qcol:qcol + 1])
                    nc.vector.tensor_tensor(out=acc[:], in0=acc[:],
                                            in1=tmp2[:], op=AL.add)

            def emit_tile(t, g):
                nbl = [int(nblk_sh[t, s4]) for s4 in range(4)]
                npos = nbl[0] + nbl[2]
                nneg = nbl[1] + nbl[3]
                acc = sp.tile([P, RU], mybir.dt.float32, tag="acc")
                tmp2 = sp.tile([P, RU], mybir.dt.float32, tag="tmp2")
                first = True
                for sgn in (0, 1):
                    ntot = npos if sgn == 0 else nneg
                    if ntot == 0:
                        continue
                    emit_sign(t, g, sgn, ntot, nbl, acc, tmp2, first)
                    first = False
                if first:
                    nc.vector.memset(acc[:], 0.0)
                nc.vector.tensor_scalar_mul(out=tmp2[:], in0=rEff[:, t, :],
                                            scalar1=qeff[:, t:t + 1])
                nc.vector.tensor_tensor(out=aggbig[:, t, :], in0=acc[:],
                                        in1=tmp2[:], op=AL.add)
            NBATCH = (NT + KT - 1) // KT
            ci = 0
            qn = 0
            off16 = 0
            for bt in range(NBATCH):
                t0, t1 = bt * KT, min((bt + 1) * KT, NT)
                bcols = int(sum(nblk_sh[t, s4] for t in range(t0, t1)
                                for s4 in range(4)))
                b16 = bcols * P // 16
                idx_t = ixp.tile([P, b16], mybir.dt.int16, tag="idx")
                nc.sync.dma_start(out=idx_t[:],
                                  in_=idxs[:, off16:off16 + b16])
                l16 = 0
                g = {}
                for s4 in range(4):
                    nb = int(sum(nblk_sh[t, s4] for t in range(t0, t1)))
                    if nb == 0:
                        continue
                    cl = calls[ci]
                    assert cl["sec"] == s4 and cl["cols"] == nb
                    ci += 1
                    gt = gpools[s4].tile([P, nb, ROWE], mybir.dt.bfloat16,
                                         tag=f"g{s4}")
                    ni = nb * P
                    nc.gpsimd.dma_gather(
                        gt[:], win(s4),
                        idx_t[:, l16:l16 + ni // 16],
                        ni, ni, ROWE,
                        single_packet=False, queue_num=qn % NQ)
                    qn += 1
                    off16 += ni // 16
                    l16 += ni // 16
                    c0 = cl["col0"]
                    chunks = []
                    h0 = 0
                    while h0 < nb:
                        hn = min((nb + 1) // 2, nb - h0)
                        m0b = mpools[s4].tile([P, hn, P], mybir.dt.bfloat16,
                                              tag=f"m{s4}")
                        nc.vector.tensor_tensor(
                            out=m0b[:],
                            in0=iota[:, None, :].broadcast_to([P, hn, P]),
                            in1=dstl_sb[:, c0 + h0:c0 + h0 + hn, None]
                            .broadcast_to([P, hn, P]),
                            op=AL.is_equal)
                        for r in range(h0, h0 + hn):
                            chunks.append((m0b, r - h0))
                        h0 += hn
                    g[s4] = (gt, c0, chunks)

                for t in range(t0, t1):
                    emit_tile(t, g)

            # ---- batched tail -------------------------------------------
            rden = cp.tile([P, NT], mybir.dt.float32)
            nc.vector.reciprocal(rden[:], aggbig[:, :, HIDDEN])
            ybig = bigp.tile([P, NT, HIDDEN], mybir.dt.float32, tag="ybig")
            nc.vector.tensor_tensor(
                out=ybig[:], in0=aggbig[:, :, 0:HIDDEN],
                in1=rden[:, :, None].broadcast_to([P, NT, HIDDEN]),
                op=AL.mult)
            nc.vector.tensor_tensor(
                out=ybig[:], in0=ybig[:],
                in1=bias_sb[:, None, :].broadcast_to([P, NT, HIDDEN]),
                op=AL.add)
            tmin = bigp.tile([P, NT, HIDDEN], mybir.dt.bfloat16, tag="tmin")
            nc.vector.tensor_scalar_min(out=tmin[:], in0=ybig[:], scalar1=0.0)
            nc.scalar.activation(out=tmin[:], in_=tmin[:], func=AF.Exp)
            nc.vector.tensor_scalar_max(out=ybig[:], in0=ybig[:], scalar1=0.0)
            nc.vector.tensor_scalar(out=tmin[:], in0=tmin[:], scalar1=1.0,
                                    scalar2=None, op0=AL.subtract)
            nc.vector.tensor_tensor(out=ybig[:], in0=ybig[:], in1=tmin[:],
                                    op=AL.add)
            for t in range(NT):
                yT_ps = pyp.tile([HIDDEN, P], mybir.dt.float32, space="PSUM",
                                 tag="yT")
                nc.tensor.transpose(out=yT_ps[:], in_=ybig[:, t, :],
                                    identity=ident[:])
                yT_sb = sp.tile([HIDDEN, P], mybir.dt.float32, tag="yT_sb")
                nc.vector.tensor_copy(out=yT_sb[:], in_=yT_ps[:])
                z_ps = pzp.tile([P, OUT_CH], mybir.dt.float32, space="PSUM",
                                tag="z")
                nc.tensor.matmul(out=z_ps[:], lhsT=yT_sb[:], rhs=linW_sb[:],
                                 start=True, stop=True)
                nc.vector.tensor_tensor(out=zst[:, t, :], in0=z_ps[:],
                                        in1=linb_sb[:], op=AL.add)
            nmx = cp.tile([P, NT], mybir.dt.float32)
            nc.vector.tensor_reduce(out=nmx[:], in_=zst[:],
                                    axis=mybir.AxisListType.X, op=AL.max)
            es = bigp.tile([P, NT, OUT_CH], mybir.dt.float32, tag="es")
            nc.vector.tensor_tensor(
                out=es[:], in0=zst[:],
                in1=nmx[:, :, None].broadcast_to([P, NT, OUT_CH]),
                op=AL.subtract)
            ex = bigp.tile([P, NT, OUT_CH], mybir.dt.bfloat16, tag="ex")
            nc.scalar.activation(out=ex[:], in_=es[:], func=AF.Exp)
            ssum = cp.tile([P, NT], mybir.dt.float32)
            nc.vector.tensor_reduce(out=ssum[:], in_=ex[:],
                                    axis=mybir.AxisListType.X, op=AL.add)
            lsum = cp.tile([P, NT], mybir.dt.float32)
            nc.scalar.activation(out=lsum[:], in_=ssum[:], func=AF.Ln)
            nc.vector.tensor_tensor(
                out=es[:], in0=es[:],
                in1=lsum[:, :, None].broadcast_to([P, NT, OUT_CH]),
                op=AL.subtract)
            nc.sync.dma_start(out=outz[:, :, :], in_=es[:])

    nc.finalize()
    return nc


EXEC_TIMES = []


def kernel(x, edge_index, W, att_src, att_dst, gat_bias, lin_W, lin_b):
    import os
    from concourse.bass_utils import run_bass_kernel_spmd
    trace = os.environ.get("GAT_TRACE") == "1"
    EXEC_TIMES.clear()

    x = np.asarray(x, _f32)
    edge_index = np.asarray(edge_index).astype(np.int64)
    W = np.asarray(W, _f32)
    att_src = np.asarray(att_src, _f32)
    att_dst = np.asarray(att_dst, _f32)
    gat_bias = np.asarray(gat_bias, _f32)
    lin_W = np.asarray(lin_W, _f32)
    lin_b = np.asarray(lin_b, _f32)

    # ---- phase A --------------------------------------------------------
    nc_a = _build_phase_a()
    xT = np.ascontiguousarray(x.T)
    att2 = np.stack([att_src, att_dst], axis=1)
    in_maps_a = []
    for c in range(CORES):
        sh = np.zeros((P, NT_A * P), _f32)
        sh[:, :NA] = xT[:, c * NA:(c + 1) * NA]
        in_maps_a.append({"xT": sh, "W": W, "att": att2})
    res_a = run_bass_kernel_spmd(nc_a, in_maps_a, core_ids=list(range(CORES)),
                                 trace=trace)
    EXEC_TIMES.append(("phase_a", res_a.exec_time_ns))

    NPAD = CORES * NT_A * P
    rowsP = np.zeros((NPAD, RU), _bf16)
    rowsM = np.zeros((NPAD, RU), _bf16)
    av = np.zeros((NPAD, 2), _f32)
    qv = np.zeros((NPAD, 2), _f32)
    for c in range(CORES):
        r = res_a.results[c]
        sl = slice(c * NA, (c + 1) * NA)
        rowsP[sl] = r["rowP"].transpose(1, 0, 2).reshape(-1, RU)[:NA]
        rowsM[sl] = r["rowM"].transpose(1, 0, 2).reshape(-1, RU)[:NA]
        av[sl] = r["avals"].transpose(1, 0, 2).reshape(-1, 2)[:NA]
        qv[sl] = r["qvals"].transpose(1, 0, 2).reshape(-1, 2)[:NA]
    rowsP, rowsM, av, qv = (rowsP[:N_NODES], rowsM[:N_NODES],
                            av[:N_NODES], qv[:N_NODES])

    tblP = np.zeros((N_NODES, ROWE), _bf16)
    tblP[:, 0:RU] = rowsP
    tblM = np.zeros((N_NODES, ROWE), _bf16)
    tblM[:, 0:RU] = rowsM

    # ---- host layout ----------------------------------------------------
    src, dst = edge_index[0], edge_index[1]
    sign = (av[src, 0] + av[dst, 1]) >= 0.0
    lay = _layout2(src, dst, sign)
    node_of = lay["node_of"]

    biasr = np.tile(gat_bias[None, :], (P, 1)).astype(_f32)
    linbr = np.tile(lin_b[None, :], (P, 1)).astype(_f32)

    in_maps_b = []
    for c in range(CORES):
        nm = node_of[c]
        nmc = np.where(nm >= 0, nm, 0)
        qstg = qv[nmc].transpose(1, 0, 2).astype(_f32)       # [P, NT, 2]
        ssign = ((av[nmc, 0] + av[nmc, 1]) >= 0.0)           # [NT, P]
        sR = np.where(ssign[:, :, None], rowsP[nmc], rowsM[nmc])
        qeffh = np.where(ssign, qv[nmc][:, :, 0], qv[nmc][:, :, 1])
        in_maps_b.append({
            "tblP": tblP, "tblM": tblM,
            "idxs": lay["idx"][c], "dstl": lay["dstl"][c],
            "qst": np.ascontiguousarray(qstg),
            "selfR": np.ascontiguousarray(sR.transpose(1, 0, 2)),
            "qeffh": np.ascontiguousarray(qeffh.T.astype(_f32)),
            "biasr": biasr, "linW": lin_W, "linbr": linbr,
        })

    nc_b = _build_phase_b(lay["calls"], lay["col_base"], lay["nblk_sh"],
                          lay["total_cols"])
    res_b = run_bass_kernel_spmd(nc_b, in_maps_b, core_ids=list(range(CORES)),
                                 trace=trace)
    EXEC_TIMES.append(("phase_b", res_b.exec_time_ns))

    out = np.zeros((N_NODES, OUT_CH), _f32)
    for c in range(CORES):
        oz = res_b.results[c]["outz"]                 # [P, NT, OUT_CH]
        nm = node_of[c]                               # [NT, P]
        valid = nm >= 0
        out[nm[valid]] = oz.transpose(1, 0, 2)[valid]
    return out
